# revision 13
# baseline (speedup 1.0000x reference)
"""Trainium2 Bass kernel for BoostedPointPairNet2.

Model (per (b, d) group, m = 128 points, din = 3):
  H1(i,j) = relu(W1A @ x_j + W1B @ x_i + b1)          (64)
  H2(i,j) = relu(W2 @ H1 + b2)                        (128)
  G(i,j)  = W3 @ H2                                    (256, b3 deferred)
  P       = max_{i,j} G + b3                           (256)
  Y       = V3 @ relu(V2 @ relu(V1 @ P + c1) + c2) + c3  (40)
  out[b]  = max_d Y[b, d]

Sharding: 16 (b, d) groups over 8 cores, 2 groups per core. Weights
replicated. Each core returns its two groups' Y rows; the host does the
final max over d (the trivial "all-gather" of a (b, 40) output).

Per-core dataflow ("stacked pairs" layout): channels of two j-values are
stacked on the 128 SBUF partitions (j even -> 0-63, j odd -> 64-127).
Engine assignment per iteration (4 j-pairs = 1024 pairs):
  PE  : H1pre as ONE K=9 matmul [w19 = [W1A_e;W1A_o;W1B_2]] against a
        host-prebuilt 9-row rhs (x_j broadcasts + tiled x_i); L2 as a
        K=64 row-split pair (tile_position (0,0)/(64,0) run concurrently
        in the array: 2x512 cols in ~216ns); L3 into two [128,1024]
        G psum tiles (ch 0:128 / 128:256).
  ACT : relu-h1 [128,512], relu-h2 [128,1024], and a share of G copies
        to fp16 SBUF. ACTIVATE is strict 1x: (FD+352)/1.2 ns.
  DVE : direct reduce_max of G tiles from PSUM (1x) + running TT-max
        (2x fp16) of the ACT-copied tiles. Carries no H1 work at all.
The G split (ACT copy+TT vs DVE direct) is tuned so ACT and DVE finish
together; PE has slack. PSUM: one shared 4-slot ring of 2-bank tiles
(8 banks); each iteration allocates exactly 4 tiles (h1ps, h2ps, g0,
g1) so every kind gets 1-iteration double buffering.
"""

import numpy as np
import ml_dtypes

import bass_rust
import concourse.bass as bass
import concourse.mybir as mybir
from concourse.tile import TileContext
from concourse.bass_utils import run_bass_kernel_spmd

BF16 = ml_dtypes.bfloat16
F32 = np.float32
DT = mybir.dt
ALU = mybir.AluOpType
AX = mybir.AxisListType
RELU = mybir.ActivationFunctionType.Relu

N_CORES = 8
B, N, DIN = 4, 512, 3
D = 4                    # boost factor
M = N // D               # 128 points per group
GROUPS_PER_CORE = 2
JP = M // 2              # 64 stacked j-pairs per group
NPAIR = JP // 4          # 16 pipeline iterations per group (4 jp each)
K9 = 3 * DIN             # contraction dim of the fused L1 matmul

# Per-group G-tile path: t = it*2 + half in 0..31 (cycled over both groups).
# True -> ACT copy + DVE running TT-max; False -> DVE direct reduce_max.
# Balance: ACT = 32*relu_h2 + 32*relu_h1 + a*copy vs DVE = (64-a)*reduce
# + a*TT  ->  a ~= 14/64.
def _copy_path(t):
    return (t % 9) in (0, 4)


# ---------------------------------------------------------------------------
# Workaround: this walrus build accepts at most ONE sync wait per instruction
# ("Too many sync wait commands"), while Tile emits up to 3. Hoist extra
# waits onto same-engine nop instructions inserted just before the offender
# (engines execute their queue in order, so the AND-wait semantics hold).
# ---------------------------------------------------------------------------
def _split_multi_waits(nc):
    seq = 0
    for fn in nc.m.functions:
        for bb in fn.blocks:
            new = []
            changed = False
            for ins in bb.instructions:
                si = ins.sync_info
                waits = list(si.on_wait) if si is not None and si.on_wait else []
                if len(waits) > 1:
                    changed = True
                    for w in waits[:-1]:
                        seq += 1
                        new.append(
                            mybir.InstNoOp(
                                name=f"I-wsplit-{seq}",
                                engine=ins.engine,
                                sync_info=bass_rust.SyncInfo(
                                    on_wait=[w], on_update=[]
                                ),
                            )
                        )
                    ins.sync_info = bass_rust.SyncInfo(
                        on_wait=[waits[-1]], on_update=list(si.on_update or [])
                    )
                new.append(ins)
            if changed:
                bb.instructions = new


# ---------------------------------------------------------------------------
# Device program
# ---------------------------------------------------------------------------
def _build_program():
    nc = bass.Bass(
        "TRN2", target_bir_lowering=False, debug=False, num_devices=N_CORES
    )

    # 9-row rhs for the fused L1 matmul, one [9, 8192] block per group:
    # col block (it, jj) rows 0:3 = x_{2jp} (bcast), 3:6 = x_{2jp+1} (bcast),
    # 6:9 = X^T tiled (i varies along the 128 cols); jp = 4*it + jj.
    xjb = nc.declare_dram_parameter(
        "xjb", [GROUPS_PER_CORE, K9, NPAIR * 512], DT.bfloat16, isOutput=False
    )
    # col 0:128 w2stack=[W2T;W2T], 128:256 w3a=W3T[:,0:128], 256:384 w3b
    wblob = nc.declare_dram_parameter("wblob", [128, 384], DT.bfloat16, isOutput=False)
    # w19 = [[W1AT,0],[0,W1AT],[W1BT,W1BT]]  (9 x 128)
    w1blob = nc.declare_dram_parameter("w1blob", [K9, 128], DT.bfloat16, isOutput=False)
    # v1t (2x512) | v2t (4x256) | v3t (2x40)
    vblob = nc.declare_dram_parameter("vblob", [128, 2128], DT.float16, isOutput=False)
    # col 0 b1st, 1 b2c, 2:4 b3_2, 4:8 c1_4, 8:10 c2_2, 10 c3 (rows 0:40)
    cblob = nc.declare_dram_parameter("cblob", [128, 11], DT.float32, isOutput=False)
    y_out = nc.declare_dram_parameter(
        "y", [GROUPS_PER_CORE, 40], DT.float32, isOutput=True
    )

    # direct-reduce column count per half per group (for racc allocation)
    ndir = [0, 0]
    for it in range(NPAIR):
        for half in range(2):
            if not _copy_path(it * 2 + half):
                ndir[half] += 1
    assert ndir[0] == ndir[1], "copy pattern must balance halves"
    ND = ndir[0]

    with TileContext(nc) as tc:
        with (
            tc.tile_pool(name="singles", bufs=1) as singles,
            tc.tile_pool(name="xjbp", bufs=1) as xjbp,
            tc.tile_pool(name="h1p", bufs=4) as h1pool,
            tc.tile_pool(name="h2p", bufs=4) as h2pool,
            tc.tile_pool(name="gcp", bufs=6) as gcpool,
            tc.tile_pool(name="raccp", bufs=2) as raccpool,
            tc.tile_pool(name="fmlp", bufs=8) as fmlp,
            tc.tile_pool(name="psum", bufs=4, space="PSUM") as psum,
        ):
            # ---- load inputs; issue order = need order (2 HWDGE queues) ----
            sb_xjbs = []
            for g in range(GROUPS_PER_CORE):
                sb_xjbs.append(
                    xjbp.tile([K9, NPAIR * 512], DT.bfloat16, name=f"sb_xjb{g}")
                )
            sb_w1 = singles.tile([K9, 128], DT.bfloat16, tag="w1blob")
            nc.scalar.dma_start(out=sb_w1, in_=w1blob[:, :])
            nc.sync.dma_start(out=sb_xjbs[0], in_=xjb[0])
            sb_c = singles.tile([128, 11], DT.float32, tag="cblob")
            nc.scalar.dma_start(out=sb_c, in_=cblob[:, :])
            sb_w = singles.tile([128, 384], DT.bfloat16, tag="wblob")
            nc.scalar.dma_start(out=sb_w, in_=wblob[:, :])
            nc.sync.dma_start(out=sb_xjbs[1], in_=xjb[1])
            sb_v = singles.tile([128, 2128], DT.float16, tag="vblob")
            nc.scalar.dma_start(out=sb_v, in_=vblob[:, :])

            # dummy relu to hoist ACT_TABLE_LOAD (~1.3us) into the init shadow
            warm = singles.tile([1, 1], DT.float32, tag="warm")
            nc.vector.memset(warm, 0.0)
            nc.scalar.activation(out=warm, in_=warm, func=RELU)

            sb_w2s = sb_w[:, 0:128]
            sb_w3a, sb_w3b = sb_w[:, 128:256], sb_w[:, 256:384]
            sb_b1st = sb_c[:, 0:1]
            sb_b2c = sb_c[:, 1:2]
            sb_b3_2 = sb_c[:, 2:4]
            sb_c1_4 = sb_c[:, 4:8]
            sb_c2_2 = sb_c[:, 8:10]
            sb_c3c = sb_c[0:40, 10:11]

            def v1t(k):  # [128, 512] fp16, k in 0..1
                return sb_v[:, 512 * k : 512 * (k + 1)]

            def v2t(k):  # [128, 256] fp16, k in 0..3
                return sb_v[:, 1024 + 256 * k : 1024 + 256 * (k + 1)]

            def v3t(k):  # [128, 40] fp16, k in 0..1
                return sb_v[:, 2048 + 40 * k : 2048 + 40 * (k + 1)]

            for g in range(GROUPS_PER_CORE):
                sb_xjb = sb_xjbs[g]
                racc = raccpool.tile([128, 2, ND], DT.float32)
                rbs = [None, None]           # running-max tiles (ch-lo, ch-hi)
                ridx = [0, 0]

                # h1 for iteration p is produced PREF iterations ahead of
                # its use so relu-h1 never sits on the current iteration's
                # critical path.
                h1tiles = {}

                def make_h1(p):
                    h1ps = psum.tile([128, 512], DT.float32, tag="ring",
                                      name=f"h1ps_{g}_{p}")
                    nc.tensor.matmul(
                        h1ps, lhsT=sb_w1,
                        rhs=sb_xjb[:, p * 512 : (p + 1) * 512],
                        start=True, stop=True,
                    )
                    h1 = h1pool.tile([128, 512], DT.bfloat16,
                                     name=f"h1_{g}_{p}", tag="h1sb")
                    nc.scalar.activation(
                        out=h1, in_=h1ps, func=RELU, bias=sb_b1st, scale=1.0
                    )
                    h1tiles[p] = h1

                PREF = 3
                for p in range(PREF):
                    make_h1(p)

                # ---- main pairwise pipeline: 4 j-pairs per iteration ----
                for it in range(NPAIR):
                    if it + PREF < NPAIR:
                        make_h1(it + PREF)
                    h1 = h1tiles.pop(it)
                    # L2: K=64 row-split pair, concurrent in the PE array
                    h2ps = psum.tile([128, 2, 512], DT.float32, tag="ring")
                    nc.tensor.matmul(
                        h2ps[:, 0, :], lhsT=sb_w2s[0:64, :], rhs=h1[0:64, :],
                        start=True, stop=True,
                    )
                    nc.tensor.matmul(
                        h2ps[:, 1, :], lhsT=sb_w2s[64:128, :], rhs=h1[64:128, :],
                        start=True, stop=True,
                    )
                    h2 = h2pool.tile([128, 1024], DT.bfloat16)
                    nc.scalar.activation(
                        out=h2, in_=h2ps.rearrange("p a b -> p (a b)"),
                        func=RELU, bias=sb_b2c, scale=1.0,
                    )
                    # L3: two [128,1024] G tiles (ch 0:128 and 128:256)
                    gts = []
                    for half, w3h in ((0, sb_w3a), (1, sb_w3b)):
                        gt = psum.tile([128, 1024], DT.float32, tag="ring")
                        nc.tensor.matmul(
                            gt[:, 0:512], lhsT=w3h, rhs=h2[:, 0:512],
                            start=True, stop=True,
                        )
                        nc.tensor.matmul(
                            gt[:, 512:1024], lhsT=w3h, rhs=h2[:, 512:1024],
                            start=True, stop=True,
                        )
                        gts.append(gt)
                    for half, gt in enumerate(gts):
                        t = it * 2 + half
                        if _copy_path(t):
                            if rbs[half] is None:
                                rb = raccpool.tile(
                                    [128, 1024], DT.float16, tag=f"rb{half}"
                                )
                                rbs[half] = rb
                                nc.scalar.copy(out=rb, in_=gt)
                            else:
                                gc = gcpool.tile([128, 1024], DT.float16)
                                nc.scalar.copy(out=gc, in_=gt)
                                nc.vector.tensor_tensor(
                                    out=rbs[half], in0=gc, in1=rbs[half], op=ALU.max
                                )
                        else:
                            k = ridx[half]
                            ridx[half] += 1
                            nc.vector.reduce_max(
                                out=racc[:, half, k : k + 1], in_=gt, axis=AX.X
                            )

                # ---- P = max over accumulators, + b3; F MLP (N=1) ----
                pm1 = fmlp.tile([128, 2], DT.float32, tag="pm1")
                nc.vector.reduce_max(out=pm1, in_=racc, axis=AX.X)
                pm2 = fmlp.tile([128, 2], DT.float32, tag="pm2")
                for half in range(2):
                    nc.vector.reduce_max(
                        out=pm2[:, half : half + 1],
                        in_=rbs[half].rearrange("p (a b) -> p a b", a=1),
                        axis=AX.X,
                    )
                pmx = fmlp.tile([128, 2], DT.float32, tag="pmx")
                nc.vector.tensor_tensor(out=pmx, in0=pm1, in1=pm2, op=ALU.max)
                pb = fmlp.tile([128, 2], DT.float16, tag="pb")
                nc.vector.tensor_tensor(out=pb, in0=pmx, in1=sb_b3_2, op=ALU.add)

                y1ps = psum.tile([128, 4], DT.float32, tag="ring")
                for mm in range(4):
                    for kk in range(2):
                        nc.tensor.matmul(
                            y1ps[:, mm : mm + 1],
                            lhsT=v1t(kk)[:, mm * 128 : (mm + 1) * 128],
                            rhs=pb[:, kk : kk + 1],
                            start=(kk == 0),
                            stop=(kk == 1),
                        )
                y1pre = fmlp.tile([128, 4], DT.float32, tag="y1pre")
                nc.vector.tensor_tensor(out=y1pre, in0=y1ps, in1=sb_c1_4, op=ALU.add)
                y1 = fmlp.tile([128, 4], DT.float16, tag="y1")
                nc.vector.tensor_scalar_max(out=y1, in0=y1pre, scalar1=0.0)

                y2ps = psum.tile([128, 2], DT.float32, tag="ring")
                for mm in range(2):
                    for kk in range(4):
                        nc.tensor.matmul(
                            y2ps[:, mm : mm + 1],
                            lhsT=v2t(kk)[:, mm * 128 : (mm + 1) * 128],
                            rhs=y1[:, kk : kk + 1],
                            start=(kk == 0),
                            stop=(kk == 3),
                        )
                y2pre = fmlp.tile([128, 2], DT.float32, tag="y2pre")
                nc.vector.tensor_tensor(out=y2pre, in0=y2ps, in1=sb_c2_2, op=ALU.add)
                y2 = fmlp.tile([128, 2], DT.float16, tag="y2")
                nc.vector.tensor_scalar_max(out=y2, in0=y2pre, scalar1=0.0)

                y3ps = psum.tile([40, 1], DT.float32, tag="ring")
                for kk in range(2):
                    nc.tensor.matmul(
                        y3ps,
                        lhsT=v3t(kk)[:, 0:40],
                        rhs=y2[:, kk : kk + 1],
                        start=(kk == 0),
                        stop=(kk == 1),
                    )
                y3 = fmlp.tile([40, 1], DT.float32, tag="y3")
                nc.vector.tensor_scalar_add(out=y3, in0=y3ps, scalar1=sb_c3c)
                nc.sync.dma_start(out=y_out[g, :], in_=y3)

    _split_multi_waits(nc)
    return nc


# ---------------------------------------------------------------------------
# Host side
# ---------------------------------------------------------------------------
_NC_CACHE = None


def _get_program():
    global _NC_CACHE
    if _NC_CACHE is None:
        _NC_CACHE = _build_program()
    return _NC_CACHE


def _make_in_maps(inputs):
    X = np.asarray(inputs["X"], F32)
    W1 = np.asarray(inputs["W1"], F32)
    b1 = np.asarray(inputs["b1"], F32)
    W2 = np.asarray(inputs["W2"], F32)
    b2 = np.asarray(inputs["b2"], F32)
    W3 = np.asarray(inputs["W3"], F32)
    b3 = np.asarray(inputs["b3"], F32)
    V1 = np.asarray(inputs["V1"], F32)
    c1 = np.asarray(inputs["c1"], F32)
    V2 = np.asarray(inputs["V2"], F32)
    c2 = np.asarray(inputs["c2"], F32)
    V3 = np.asarray(inputs["V3"], F32)
    c3 = np.asarray(inputs["c3"], F32)

    W1A, W1B = W1[:, :DIN], W1[:, DIN:]
    z = np.zeros((DIN, 64), F32)
    w1blob = np.concatenate(
        [
            np.concatenate([W1A.T, z], axis=1),
            np.concatenate([z, W1A.T], axis=1),
            np.concatenate([W1B.T, W1B.T], axis=1),
        ],
        axis=0,
    ).astype(BF16)
    wblob = np.concatenate(
        [
            np.concatenate([W2.T, W2.T], axis=0),
            W3.T[:, 0:128],
            W3.T[:, 128:256],
        ],
        axis=1,
    ).astype(BF16)
    # v1t: V1.T is [256, 512] -> k-tiles stacked on cols [128, 2, 512]
    v1t_cols = V1.T.reshape(2, 128, 512).transpose(1, 0, 2).reshape(128, 1024)
    vblob = np.concatenate(
        [v1t_cols,
         V2.T.reshape(4, 128, 256).transpose(1, 0, 2).reshape(128, 1024),
         V3.T.reshape(2, 128, 40).transpose(1, 0, 2).reshape(128, 80)],
        axis=1,
    ).astype(np.float16)
    cblob = np.zeros((128, 11), F32)
    cblob[:, 0] = np.concatenate([b1, b1])
    cblob[:, 1] = b2
    cblob[:, 2:4] = b3.reshape(2, 128).T
    cblob[:, 4:8] = c1.reshape(4, 128).T
    cblob[:, 8:10] = c2.reshape(2, 128).T
    cblob[0:40, 10] = c3

    shared = dict(wblob=wblob, w1blob=w1blob, vblob=vblob, cblob=cblob)

    Xv = X.reshape(B, D, M, DIN)                    # (b, d, i, k)
    in_maps = []
    for c in range(N_CORES):
        xjbs = np.empty((GROUPS_PER_CORE, K9, NPAIR * 512), F32)
        for gi in range(GROUPS_PER_CORE):
            g = 2 * c + gi
            bb, dd = g // D, g % D
            xg = Xv[bb, dd]                          # (M, 3)
            # per jp block of 128 cols: rows 0:3 = x_{2jp}, 3:6 = x_{2jp+1}
            blk = xjbs[gi].reshape(K9, JP, M)
            blk[0:3] = xg[0::2].T[:, :, None]        # (3, JP, 1) -> bcast
            blk[3:6] = xg[1::2].T[:, :, None]
            blk[6:9] = xg.T[:, None, :]              # (3, M) tiled over jp
        in_maps.append(dict(shared, xjb=xjbs.astype(BF16)))
    return in_maps


def _run(inputs, trace=False):
    nc = _get_program()
    in_maps = _make_in_maps(inputs)
    res = run_bass_kernel_spmd(nc, in_maps, list(range(N_CORES)), trace=trace)
    ys = np.stack([res.results[c]["y"] for c in range(N_CORES)])  # [8, 2, 40]
    y16 = ys.reshape(B, D, 40)
    out = y16.max(axis=1).astype(F32)
    return out, res


def kernel(**inputs):
    out, _ = _run(inputs, trace=False)
    return out


# revision 15
# speedup vs baseline: 1.1642x; 1.1642x over previous
"""Trainium2 Bass kernel for BoostedPointPairNet2.

Model (per (b, d) group, m = 128 points, din = 3):
  H1(i,j) = relu(W1A @ x_j + W1B @ x_i + b1)          (64)
  H2(i,j) = relu(W2 @ H1 + b2)                        (128)
  G(i,j)  = W3 @ H2                                    (256, b3 deferred)
  P       = max_{i,j} G + b3                           (256)
  Y       = V3 @ relu(V2 @ relu(V1 @ P + c1) + c2) + c3  (40)
  out[b]  = max_d Y[b, d]

Sharding: 16 (b, d) groups over 8 cores, 2 groups per core. Weights
replicated. Each core returns its two groups' Y rows; the host does the
final max over d (the trivial "all-gather" of a (b, 40) output).

Per-core dataflow ("stacked pairs" layout): channels of two j-values are
stacked on the 128 SBUF partitions (j even -> 0-63, j odd -> 64-127).
Engine assignment per iteration (4 j-pairs = 1024 pairs):
  PE  : H1pre as ONE K=9 matmul [w19 = [W1A_e;W1A_o;W1B_2]] against a
        host-prebuilt 9-row rhs (x_j broadcasts + tiled x_i); L2 as a
        K=64 row-split pair (tile_position (0,0)/(64,0) run concurrently
        in the array: 2x512 cols in ~216ns); L3 into two [128,1024]
        G psum tiles (ch 0:128 / 128:256).
  ACT : relu-h1 [128,512], relu-h2 [128,1024], and a share of G copies
        to fp16 SBUF. ACTIVATE is strict 1x: (FD+352)/1.2 ns.
  DVE : direct reduce_max of G tiles from PSUM (1x) + running TT-max
        (2x fp16) of the ACT-copied tiles. Carries no H1 work at all.
The G split (ACT copy+TT vs DVE direct) is tuned so ACT and DVE finish
together; PE has slack. PSUM: one shared 4-slot ring of 2-bank tiles
(8 banks); each iteration allocates exactly 4 tiles (h1ps, h2ps, g0,
g1) so every kind gets 1-iteration double buffering.
"""

import numpy as np
import ml_dtypes

import bass_rust
import concourse.bass as bass
import concourse.mybir as mybir
from concourse.tile import TileContext
from concourse.bass_utils import run_bass_kernel_spmd

BF16 = ml_dtypes.bfloat16
F32 = np.float32
DT = mybir.dt
ALU = mybir.AluOpType
AX = mybir.AxisListType
RELU = mybir.ActivationFunctionType.Relu

N_CORES = 8
B, N, DIN = 4, 512, 3
D = 4                    # boost factor
M = N // D               # 128 points per group
GROUPS_PER_CORE = 2
JP = M // 2              # 64 stacked j-pairs per group
NPAIR = JP // 4          # 16 pipeline iterations per group (4 jp each)
K9 = 3 * DIN             # contraction dim of the fused L1 matmul

# Per-group G-tile path: t = it*4 + q in 0..63 (q = L3 quarter: ch-half
# = q>>1, pair-part = q&1; each G tile is [128, 512] fp32 in PSUM).
# True -> ACT copy + DVE running TT-max; False -> DVE direct reduce_max.
# Balance: ACT = relu_h2 + relu_h1/2 + a*copy vs DVE = (128-a)*reduce +
# a*TT (per core)  ->  a ~= 36/128 -> 18/64 per group.
def _copy_path(t):
    return (t % 7) in (1, 4)


# ---------------------------------------------------------------------------
# Workaround: this walrus build accepts at most ONE sync wait per instruction
# ("Too many sync wait commands"), while Tile emits up to 3. Hoist extra
# waits onto same-engine nop instructions inserted just before the offender
# (engines execute their queue in order, so the AND-wait semantics hold).
# ---------------------------------------------------------------------------
def _split_multi_waits(nc):
    seq = 0
    for fn in nc.m.functions:
        for bb in fn.blocks:
            new = []
            changed = False
            for ins in bb.instructions:
                si = ins.sync_info
                waits = list(si.on_wait) if si is not None and si.on_wait else []
                if len(waits) > 1:
                    changed = True
                    for w in waits[:-1]:
                        seq += 1
                        new.append(
                            mybir.InstNoOp(
                                name=f"I-wsplit-{seq}",
                                engine=ins.engine,
                                sync_info=bass_rust.SyncInfo(
                                    on_wait=[w], on_update=[]
                                ),
                            )
                        )
                    ins.sync_info = bass_rust.SyncInfo(
                        on_wait=[waits[-1]], on_update=list(si.on_update or [])
                    )
                new.append(ins)
            if changed:
                bb.instructions = new


# ---------------------------------------------------------------------------
# Device program
# ---------------------------------------------------------------------------
def _build_program():
    nc = bass.Bass(
        "TRN2", target_bir_lowering=False, debug=False, num_devices=N_CORES
    )

    # 9-row rhs for the fused L1 matmul, one [9, 8192] block per group:
    # col block (it, jj) rows 0:3 = x_{2jp} (bcast), 3:6 = x_{2jp+1} (bcast),
    # 6:9 = X^T tiled (i varies along the 128 cols); jp = 4*it + jj.
    xjb = nc.declare_dram_parameter(
        "xjb", [GROUPS_PER_CORE, K9, NPAIR * 512], DT.bfloat16, isOutput=False
    )
    # col 0:128 w2stack=[W2T;W2T], 128:256 w3a=W3T[:,0:128], 256:384 w3b
    wblob = nc.declare_dram_parameter("wblob", [128, 384], DT.bfloat16, isOutput=False)
    # w19 = [[W1AT,0],[0,W1AT],[W1BT,W1BT]]  (9 x 128)
    w1blob = nc.declare_dram_parameter("w1blob", [K9, 128], DT.bfloat16, isOutput=False)
    # v1t (2x512) | v2t (4x256) | v3t (2x40)
    vblob = nc.declare_dram_parameter("vblob", [128, 2128], DT.float16, isOutput=False)
    # col 0 b1st, 1 b2c, 2:4 b3_2, 4:8 c1_4, 8:10 c2_2, 10 c3 (rows 0:40)
    cblob = nc.declare_dram_parameter("cblob", [128, 11], DT.float32, isOutput=False)
    y_out = nc.declare_dram_parameter(
        "y", [GROUPS_PER_CORE, 40], DT.float32, isOutput=True
    )

    # direct-reduce column count per half per group (for racc allocation);
    # racc is memset to -1e30 so unequal halves are safe.
    ndir = [0, 0]
    for it in range(NPAIR):
        for q in range(4):
            if not _copy_path(it * 4 + q):
                ndir[q >> 1] += 1
    ND = max(ndir)

    with TileContext(nc) as tc:
        with (
            tc.tile_pool(name="singles", bufs=1) as singles,
            tc.tile_pool(name="xjbp", bufs=1) as xjbp,
            tc.tile_pool(name="h1p", bufs=4) as h1pool,
            tc.tile_pool(name="h2p", bufs=4) as h2pool,
            tc.tile_pool(name="gcp", bufs=6) as gcpool,
            tc.tile_pool(name="raccp", bufs=2) as raccpool,
            tc.tile_pool(name="fmlp", bufs=8) as fmlp,
            tc.tile_pool(name="ps_h1", bufs=1, space="PSUM") as ps_h1,
            tc.tile_pool(name="ps_l2", bufs=2, space="PSUM") as ps_l2,
            tc.tile_pool(name="ps_g", bufs=2, space="PSUM") as ps_g,
        ):
            # ---- load inputs; issue order = need order (2 HWDGE queues) ----
            sb_xjbs = []
            for g in range(GROUPS_PER_CORE):
                sb_xjbs.append(
                    xjbp.tile([K9, NPAIR * 512], DT.bfloat16, name=f"sb_xjb{g}")
                )
            sb_w1 = singles.tile([K9, 128], DT.bfloat16, tag="w1blob")
            nc.scalar.dma_start(out=sb_w1, in_=w1blob[:, :])
            nc.sync.dma_start(out=sb_xjbs[0], in_=xjb[0])
            sb_c = singles.tile([128, 11], DT.float32, tag="cblob")
            nc.scalar.dma_start(out=sb_c, in_=cblob[:, :])
            sb_w = singles.tile([128, 384], DT.bfloat16, tag="wblob")
            nc.scalar.dma_start(out=sb_w, in_=wblob[:, :])
            nc.sync.dma_start(out=sb_xjbs[1], in_=xjb[1])
            sb_v = singles.tile([128, 2128], DT.float16, tag="vblob")
            nc.scalar.dma_start(out=sb_v, in_=vblob[:, :])

            # dummy relu to hoist ACT_TABLE_LOAD (~1.3us) into the init shadow
            warm = singles.tile([1, 1], DT.float32, tag="warm")
            nc.vector.memset(warm, 0.0)
            nc.scalar.activation(out=warm, in_=warm, func=RELU)

            sb_w2s = sb_w[:, 0:128]
            sb_w3a, sb_w3b = sb_w[:, 128:256], sb_w[:, 256:384]
            sb_b1st = sb_c[:, 0:1]
            sb_b2c = sb_c[:, 1:2]
            sb_b3_2 = sb_c[:, 2:4]
            sb_c1_4 = sb_c[:, 4:8]
            sb_c2_2 = sb_c[:, 8:10]
            sb_c3c = sb_c[0:40, 10:11]

            def v1t(k):  # [128, 512] fp16, k in 0..1
                return sb_v[:, 512 * k : 512 * (k + 1)]

            def v2t(k):  # [128, 256] fp16, k in 0..3
                return sb_v[:, 1024 + 256 * k : 1024 + 256 * (k + 1)]

            def v3t(k):  # [128, 40] fp16, k in 0..1
                return sb_v[:, 2048 + 40 * k : 2048 + 40 * (k + 1)]

            # ---- PE warm-up burst: ~12 back-to-back MMs during the DMA
            # phase so HAM unthrottles (1.2 -> 2.4 GHz) before the pipeline.
            scratch = singles.tile([128, 512], DT.bfloat16, tag="scratch")
            nc.vector.memset(scratch, 0.0)
            wps = ps_l2.tile([128, 2, 512], DT.float32, tag="l2", name="wps")
            for rep in range(12):
                nc.tensor.matmul(
                    wps[:, rep % 2, :], lhsT=scratch[:, 0:128], rhs=scratch,
                    start=True, stop=True,
                )

            for g in range(GROUPS_PER_CORE):
                sb_xjb = sb_xjbs[g]
                racc = raccpool.tile([128, 2, ND], DT.float32)
                nc.vector.memset(racc, -1e30)
                rbs = [None, None]           # running-max tiles (ch-lo, ch-hi)
                ridx = [0, 0]

                # h1 in 2-iteration pairs, prefetched PP pairs ahead; L2+relu
                # run one iteration ahead of L3/consume (software pipeline).
                h1tiles = {}

                def make_h1_pair(p):
                    h1ps = ps_h1.tile([128, 1024], DT.float32, tag="h1",
                                      name=f"h1ps_{g}_{p}")
                    nc.tensor.matmul(
                        h1ps[:, 0:512], lhsT=sb_w1,
                        rhs=sb_xjb[:, (2 * p) * 512 : (2 * p + 1) * 512],
                        start=True, stop=True,
                    )
                    nc.tensor.matmul(
                        h1ps[:, 512:1024], lhsT=sb_w1,
                        rhs=sb_xjb[:, (2 * p + 1) * 512 : (2 * p + 2) * 512],
                        start=True, stop=True,
                    )
                    h1 = h1pool.tile([128, 1024], DT.bfloat16,
                                     name=f"h1_{g}_{p}", tag="h1sb")
                    nc.scalar.activation(
                        out=h1, in_=h1ps, func=RELU, bias=sb_b1st, scale=1.0
                    )
                    h1tiles[p] = h1

                h2tiles = {}

                def make_h2(it):
                    h1 = h1tiles[it // 2][:, (it % 2) * 512 : (it % 2 + 1) * 512]
                    h2ps = ps_l2.tile([128, 2, 512], DT.float32, tag="l2",
                                      name=f"h2ps_{g}_{it}")
                    nc.tensor.matmul(
                        h2ps[:, 0, :], lhsT=sb_w2s[0:64, :], rhs=h1[0:64, :],
                        start=True, stop=True,
                    )
                    nc.tensor.matmul(
                        h2ps[:, 1, :], lhsT=sb_w2s[64:128, :], rhs=h1[64:128, :],
                        start=True, stop=True,
                    )
                    h2 = h2pool.tile([128, 1024], DT.bfloat16,
                                     name=f"h2_{g}_{it}", tag="h2sb")
                    nc.scalar.activation(
                        out=h2, in_=h2ps.rearrange("p a b -> p (a b)"),
                        func=RELU, bias=sb_b2c, scale=1.0,
                    )
                    h2tiles[it] = h2

                PP = 2
                for p in range(PP):
                    make_h1_pair(p)
                make_h2(0)

                # ---- main pipeline: per iteration, L2/relu for it+1 then
                # L3 + G-consumption for it (G streams via 2 psum banks) ----
                for it in range(NPAIR):
                    if it % 2 == 0 and it // 2 + PP < NPAIR // 2:
                        make_h1_pair(it // 2 + PP)
                    if it + 1 < NPAIR:
                        make_h2(it + 1)
                    h2 = h2tiles.pop(it)
                    for q in range(4):
                        w3h = (sb_w3a, sb_w3a, sb_w3b, sb_w3b)[q]
                        half = q >> 1
                        part = q & 1
                        gt = ps_g.tile([128, 512], DT.float32, tag="g",
                                       name=f"gt_{g}_{it}_{q}")
                        nc.tensor.matmul(
                            gt, lhsT=w3h,
                            rhs=h2[:, part * 512 : (part + 1) * 512],
                            start=True, stop=True,
                        )
                        t = it * 4 + q
                        if _copy_path(t):
                            if rbs[half] is None:
                                rb = raccpool.tile(
                                    [128, 512], DT.float16, tag=f"rb{half}"
                                )
                                rbs[half] = rb
                                nc.scalar.copy(out=rb, in_=gt)
                            else:
                                gc = gcpool.tile([128, 512], DT.float16)
                                nc.scalar.copy(out=gc, in_=gt)
                                nc.vector.tensor_tensor(
                                    out=rbs[half], in0=gc, in1=rbs[half],
                                    op=ALU.max,
                                )
                        else:
                            k = ridx[half]
                            ridx[half] += 1
                            nc.vector.reduce_max(
                                out=racc[:, half, k : k + 1], in_=gt, axis=AX.X
                            )

                # ---- P = max over accumulators, + b3; F MLP (N=1) ----
                pm1 = fmlp.tile([128, 2], DT.float32, tag="pm1")
                nc.vector.reduce_max(out=pm1, in_=racc, axis=AX.X)
                pm2 = fmlp.tile([128, 2], DT.float32, tag="pm2")
                for half in range(2):
                    nc.vector.reduce_max(
                        out=pm2[:, half : half + 1],
                        in_=rbs[half].rearrange("p (a b) -> p a b", a=1),
                        axis=AX.X,
                    )
                pmx = fmlp.tile([128, 2], DT.float32, tag="pmx")
                nc.vector.tensor_tensor(out=pmx, in0=pm1, in1=pm2, op=ALU.max)
                pb = fmlp.tile([128, 2], DT.float16, tag="pb")
                nc.vector.tensor_tensor(out=pb, in0=pmx, in1=sb_b3_2, op=ALU.add)

                y1ps = ps_l2.tile([128, 4], DT.float32, tag="l2",
                                  name=f"y1ps_{g}")
                for mm in range(4):
                    for kk in range(2):
                        nc.tensor.matmul(
                            y1ps[:, mm : mm + 1],
                            lhsT=v1t(kk)[:, mm * 128 : (mm + 1) * 128],
                            rhs=pb[:, kk : kk + 1],
                            start=(kk == 0),
                            stop=(kk == 1),
                        )
                y1pre = fmlp.tile([128, 4], DT.float32, tag="y1pre")
                nc.vector.tensor_tensor(out=y1pre, in0=y1ps, in1=sb_c1_4, op=ALU.add)
                y1 = fmlp.tile([128, 4], DT.float16, tag="y1")
                nc.vector.tensor_scalar_max(out=y1, in0=y1pre, scalar1=0.0)

                y2ps = ps_h1.tile([128, 2], DT.float32, tag="h1",
                                  name=f"y2ps_{g}")
                for mm in range(2):
                    for kk in range(4):
                        nc.tensor.matmul(
                            y2ps[:, mm : mm + 1],
                            lhsT=v2t(kk)[:, mm * 128 : (mm + 1) * 128],
                            rhs=y1[:, kk : kk + 1],
                            start=(kk == 0),
                            stop=(kk == 3),
                        )
                y2pre = fmlp.tile([128, 2], DT.float32, tag="y2pre")
                nc.vector.tensor_tensor(out=y2pre, in0=y2ps, in1=sb_c2_2, op=ALU.add)
                y2 = fmlp.tile([128, 2], DT.float16, tag="y2")
                nc.vector.tensor_scalar_max(out=y2, in0=y2pre, scalar1=0.0)

                y3ps = ps_g.tile([40, 1], DT.float32, tag="g",
                                 name=f"y3ps_{g}")
                for kk in range(2):
                    nc.tensor.matmul(
                        y3ps,
                        lhsT=v3t(kk)[:, 0:40],
                        rhs=y2[:, kk : kk + 1],
                        start=(kk == 0),
                        stop=(kk == 1),
                    )
                y3 = fmlp.tile([40, 1], DT.float32, tag="y3")
                nc.vector.tensor_scalar_add(out=y3, in0=y3ps, scalar1=sb_c3c)
                nc.sync.dma_start(out=y_out[g, :], in_=y3)

    _split_multi_waits(nc)
    return nc


# ---------------------------------------------------------------------------
# Host side
# ---------------------------------------------------------------------------
_NC_CACHE = None


def _get_program():
    global _NC_CACHE
    if _NC_CACHE is None:
        _NC_CACHE = _build_program()
    return _NC_CACHE


def _make_in_maps(inputs):
    X = np.asarray(inputs["X"], F32)
    W1 = np.asarray(inputs["W1"], F32)
    b1 = np.asarray(inputs["b1"], F32)
    W2 = np.asarray(inputs["W2"], F32)
    b2 = np.asarray(inputs["b2"], F32)
    W3 = np.asarray(inputs["W3"], F32)
    b3 = np.asarray(inputs["b3"], F32)
    V1 = np.asarray(inputs["V1"], F32)
    c1 = np.asarray(inputs["c1"], F32)
    V2 = np.asarray(inputs["V2"], F32)
    c2 = np.asarray(inputs["c2"], F32)
    V3 = np.asarray(inputs["V3"], F32)
    c3 = np.asarray(inputs["c3"], F32)

    W1A, W1B = W1[:, :DIN], W1[:, DIN:]
    z = np.zeros((DIN, 64), F32)
    w1blob = np.concatenate(
        [
            np.concatenate([W1A.T, z], axis=1),
            np.concatenate([z, W1A.T], axis=1),
            np.concatenate([W1B.T, W1B.T], axis=1),
        ],
        axis=0,
    ).astype(BF16)
    wblob = np.concatenate(
        [
            np.concatenate([W2.T, W2.T], axis=0),
            W3.T[:, 0:128],
            W3.T[:, 128:256],
        ],
        axis=1,
    ).astype(BF16)
    # v1t: V1.T is [256, 512] -> k-tiles stacked on cols [128, 2, 512]
    v1t_cols = V1.T.reshape(2, 128, 512).transpose(1, 0, 2).reshape(128, 1024)
    vblob = np.concatenate(
        [v1t_cols,
         V2.T.reshape(4, 128, 256).transpose(1, 0, 2).reshape(128, 1024),
         V3.T.reshape(2, 128, 40).transpose(1, 0, 2).reshape(128, 80)],
        axis=1,
    ).astype(np.float16)
    cblob = np.zeros((128, 11), F32)
    cblob[:, 0] = np.concatenate([b1, b1])
    cblob[:, 1] = b2
    cblob[:, 2:4] = b3.reshape(2, 128).T
    cblob[:, 4:8] = c1.reshape(4, 128).T
    cblob[:, 8:10] = c2.reshape(2, 128).T
    cblob[0:40, 10] = c3

    shared = dict(wblob=wblob, w1blob=w1blob, vblob=vblob, cblob=cblob)

    Xv = X.reshape(B, D, M, DIN)                    # (b, d, i, k)
    in_maps = []
    for c in range(N_CORES):
        xjbs = np.empty((GROUPS_PER_CORE, K9, NPAIR * 512), F32)
        for gi in range(GROUPS_PER_CORE):
            g = 2 * c + gi
            bb, dd = g // D, g % D
            xg = Xv[bb, dd]                          # (M, 3)
            # per jp block of 128 cols: rows 0:3 = x_{2jp}, 3:6 = x_{2jp+1}
            blk = xjbs[gi].reshape(K9, JP, M)
            blk[0:3] = xg[0::2].T[:, :, None]        # (3, JP, 1) -> bcast
            blk[3:6] = xg[1::2].T[:, :, None]
            blk[6:9] = xg.T[:, None, :]              # (3, M) tiled over jp
        in_maps.append(dict(shared, xjb=xjbs.astype(BF16)))
    return in_maps


def _run(inputs, trace=False):
    nc = _get_program()
    in_maps = _make_in_maps(inputs)
    res = run_bass_kernel_spmd(nc, in_maps, list(range(N_CORES)), trace=trace)
    ys = np.stack([res.results[c]["y"] for c in range(N_CORES)])  # [8, 2, 40]
    y16 = ys.reshape(B, D, 40)
    out = y16.max(axis=1).astype(F32)
    return out, res


def kernel(**inputs):
    out, _ = _run(inputs, trace=False)
    return out


# revision 16
# speedup vs baseline: 1.4177x; 1.2178x over previous
"""Trainium2 Bass kernel for BoostedPointPairNet2.

Model (per (b, d) group, m = 128 points, din = 3):
  H1(i,j) = relu(W1A @ x_j + W1B @ x_i + b1)          (64)
  H2(i,j) = relu(W2 @ H1 + b2)                        (128)
  G(i,j)  = W3 @ H2                                    (256, b3 deferred)
  P       = max_{i,j} G + b3                           (256)
  Y       = V3 @ relu(V2 @ relu(V1 @ P + c1) + c2) + c3  (40)
  out[b]  = max_d Y[b, d]

Sharding: 16 (b, d) groups over 8 cores, 2 groups per core. Weights
replicated. Each core returns its two groups' Y rows; the host does the
final max over d (the trivial "all-gather" of a (b, 40) output).

Per-core dataflow ("stacked pairs" layout): channels of two j-values are
stacked on the 128 SBUF partitions (j even -> 0-63, j odd -> 64-127).
Engine assignment per iteration (4 j-pairs = 1024 pairs):
  PE  : H1pre as ONE K=9 matmul [w19 = [W1A_e;W1A_o;W1B_2]] against a
        host-prebuilt 9-row rhs (x_j broadcasts + tiled x_i); L2 as a
        K=64 row-split pair (tile_position (0,0)/(64,0) run concurrently
        in the array: 2x512 cols in ~216ns); L3 into two [128,1024]
        G psum tiles (ch 0:128 / 128:256).
  ACT : relu-h1 [128,512], relu-h2 [128,1024], and a share of G copies
        to fp16 SBUF. ACTIVATE is strict 1x: (FD+352)/1.2 ns.
  DVE : direct reduce_max of G tiles from PSUM (1x) + running TT-max
        (2x fp16) of the ACT-copied tiles. Carries no H1 work at all.
The G split (ACT copy+TT vs DVE direct) is tuned so ACT and DVE finish
together; PE has slack. PSUM: one shared 4-slot ring of 2-bank tiles
(8 banks); each iteration allocates exactly 4 tiles (h1ps, h2ps, g0,
g1) so every kind gets 1-iteration double buffering.
"""

import numpy as np
import ml_dtypes

import bass_rust
import concourse.bass as bass
import concourse.mybir as mybir
from concourse.tile import TileContext
from concourse.bass_utils import run_bass_kernel_spmd

BF16 = ml_dtypes.bfloat16
F32 = np.float32
DT = mybir.dt
ALU = mybir.AluOpType
AX = mybir.AxisListType
RELU = mybir.ActivationFunctionType.Relu

N_CORES = 8
B, N, DIN = 4, 512, 3
D = 4                    # boost factor
M = N // D               # 128 points per group
GROUPS_PER_CORE = 2
JP = M // 2              # 64 stacked j-pairs per group
NPAIR = JP // 4          # 16 pipeline iterations per group (4 jp each)
K9 = 3 * DIN             # contraction dim of the fused L1 matmul

# Per-group G-tile path: t = it*2 + half in 0..31 (half: 0 = channels
# 0:128 via w3a, 1 = 128:256 via w3b; each G tile is [128, 1024] fp32).
# True -> ACT copy + DVE running TT-max; False -> DVE direct reduce_max.
# Balance: ACT = relu_h2 + relu_h1/2 + a*copy vs DVE = (64-a)*reduce +
# a*TT (per core)  ->  a ~= 18/64 -> 9/32 per group.
def _copy_path(t):
    return (t % 7) in (1, 4)


# ---------------------------------------------------------------------------
# Workaround: this walrus build accepts at most ONE sync wait per instruction
# ("Too many sync wait commands"), while Tile emits up to 3. Hoist extra
# waits onto same-engine nop instructions inserted just before the offender
# (engines execute their queue in order, so the AND-wait semantics hold).
# ---------------------------------------------------------------------------
def _split_multi_waits(nc):
    seq = 0
    for fn in nc.m.functions:
        for bb in fn.blocks:
            new = []
            changed = False
            for ins in bb.instructions:
                si = ins.sync_info
                waits = list(si.on_wait) if si is not None and si.on_wait else []
                if len(waits) > 1:
                    changed = True
                    for w in waits[:-1]:
                        seq += 1
                        new.append(
                            mybir.InstNoOp(
                                name=f"I-wsplit-{seq}",
                                engine=ins.engine,
                                sync_info=bass_rust.SyncInfo(
                                    on_wait=[w], on_update=[]
                                ),
                            )
                        )
                    ins.sync_info = bass_rust.SyncInfo(
                        on_wait=[waits[-1]], on_update=list(si.on_update or [])
                    )
                new.append(ins)
            if changed:
                bb.instructions = new


# ---------------------------------------------------------------------------
# Device program
# ---------------------------------------------------------------------------
def _build_program():
    nc = bass.Bass(
        "TRN2", target_bir_lowering=False, debug=False, num_devices=N_CORES
    )

    # 9-row rhs for the fused L1 matmul, one [9, 8192] block per group:
    # col block (it, jj) rows 0:3 = x_{2jp} (bcast), 3:6 = x_{2jp+1} (bcast),
    # 6:9 = X^T tiled (i varies along the 128 cols); jp = 4*it + jj.
    xjb = nc.declare_dram_parameter(
        "xjb", [GROUPS_PER_CORE, K9, NPAIR * 512], DT.bfloat16, isOutput=False
    )
    # col 0:128 w2stack=[W2T;W2T], 128:256 w3a=W3T[:,0:128], 256:384 w3b
    wblob = nc.declare_dram_parameter("wblob", [128, 384], DT.bfloat16, isOutput=False)
    # w19 = [[W1AT,0],[0,W1AT],[W1BT,W1BT]]  (9 x 128)
    w1blob = nc.declare_dram_parameter("w1blob", [K9, 128], DT.bfloat16, isOutput=False)
    # v1t (2x512) | v2t (4x256) | v3t (2x40)
    vblob = nc.declare_dram_parameter("vblob", [128, 2128], DT.float16, isOutput=False)
    # col 0 b1st, 1 b2c, 2:4 b3_2, 4:8 c1_4, 8:10 c2_2, 10 c3 (rows 0:40)
    cblob = nc.declare_dram_parameter("cblob", [128, 11], DT.float32, isOutput=False)
    y_out = nc.declare_dram_parameter(
        "y", [GROUPS_PER_CORE, 40], DT.float32, isOutput=True
    )

    # direct-reduce column count per half per group (for racc allocation);
    # racc is memset to -1e30 so unequal halves are safe.
    ndir = [0, 0]
    for it in range(NPAIR):
        for half in range(2):
            if not _copy_path(it * 2 + half):
                ndir[half] += 1
    ND = max(ndir)

    with TileContext(nc) as tc:
        with (
            tc.tile_pool(name="singles", bufs=1) as singles,
            tc.tile_pool(name="xjbp", bufs=1) as xjbp,
            tc.tile_pool(name="h1p", bufs=4) as h1pool,
            tc.tile_pool(name="h2p", bufs=4) as h2pool,
            tc.tile_pool(name="gcp", bufs=6) as gcpool,
            tc.tile_pool(name="raccp", bufs=2) as raccpool,
            tc.tile_pool(name="fmlp", bufs=8) as fmlp,
            tc.tile_pool(name="ps_h1", bufs=1, space="PSUM") as ps_h1,
            tc.tile_pool(name="ps_l2", bufs=1, space="PSUM") as ps_l2,
            tc.tile_pool(name="ps_g", bufs=2, space="PSUM") as ps_g,
        ):
            # ---- load inputs; issue order = need order (2 HWDGE queues) ----
            sb_xjbs = []
            for g in range(GROUPS_PER_CORE):
                sb_xjbs.append(
                    xjbp.tile([K9, NPAIR * 512], DT.bfloat16, name=f"sb_xjb{g}")
                )
            sb_w1 = singles.tile([K9, 128], DT.bfloat16, tag="w1blob")
            nc.scalar.dma_start(out=sb_w1, in_=w1blob[:, :])
            nc.sync.dma_start(out=sb_xjbs[0], in_=xjb[0])
            sb_c = singles.tile([128, 11], DT.float32, tag="cblob")
            nc.scalar.dma_start(out=sb_c, in_=cblob[:, :])
            sb_w = singles.tile([128, 384], DT.bfloat16, tag="wblob")
            nc.scalar.dma_start(out=sb_w, in_=wblob[:, :])
            nc.sync.dma_start(out=sb_xjbs[1], in_=xjb[1])
            sb_v = singles.tile([128, 2128], DT.float16, tag="vblob")
            nc.scalar.dma_start(out=sb_v, in_=vblob[:, :])

            # dummy relu to hoist ACT_TABLE_LOAD (~1.3us) into the init shadow
            warm = singles.tile([1, 1], DT.float32, tag="warm")
            nc.vector.memset(warm, 0.0)
            nc.scalar.activation(out=warm, in_=warm, func=RELU)

            sb_w2s = sb_w[:, 0:128]
            sb_w3a, sb_w3b = sb_w[:, 128:256], sb_w[:, 256:384]
            sb_b1st = sb_c[:, 0:1]
            sb_b2c = sb_c[:, 1:2]
            sb_b3_2 = sb_c[:, 2:4]
            sb_c1_4 = sb_c[:, 4:8]
            sb_c2_2 = sb_c[:, 8:10]
            sb_c3c = sb_c[0:40, 10:11]

            def v1t(k):  # [128, 512] fp16, k in 0..1
                return sb_v[:, 512 * k : 512 * (k + 1)]

            def v2t(k):  # [128, 256] fp16, k in 0..3
                return sb_v[:, 1024 + 256 * k : 1024 + 256 * (k + 1)]

            def v3t(k):  # [128, 40] fp16, k in 0..1
                return sb_v[:, 2048 + 40 * k : 2048 + 40 * (k + 1)]

            # ---- PE warm-up burst: ~12 back-to-back MMs during the DMA
            # phase so HAM unthrottles (1.2 -> 2.4 GHz) before the pipeline.
            scratch = singles.tile([128, 512], DT.bfloat16, tag="scratch")
            nc.vector.memset(scratch, 0.0)
            wps = ps_l2.tile([128, 2, 512], DT.float32, tag="l2", name="wps")
            for rep in range(12):
                nc.tensor.matmul(
                    wps[:, rep % 2, :], lhsT=scratch[:, 0:128], rhs=scratch,
                    start=True, stop=True,
                )

            for g in range(GROUPS_PER_CORE):
                sb_xjb = sb_xjbs[g]
                racc = raccpool.tile([128, 2, ND], DT.float32)
                nc.vector.memset(racc, -1e30)
                rbs = [None, None]           # running-max tiles (ch-lo, ch-hi)
                ridx = [0, 0]

                # h1 in 2-iteration pairs, prefetched PP pairs ahead; L2+relu
                # run one iteration ahead of L3/consume (software pipeline).
                h1tiles = {}

                def make_h1_pair(p):
                    h1ps = ps_h1.tile([128, 1024], DT.float32, tag="h1",
                                      name=f"h1ps_{g}_{p}")
                    nc.tensor.matmul(
                        h1ps[:, 0:512], lhsT=sb_w1,
                        rhs=sb_xjb[:, (2 * p) * 512 : (2 * p + 1) * 512],
                        start=True, stop=True,
                    )
                    nc.tensor.matmul(
                        h1ps[:, 512:1024], lhsT=sb_w1,
                        rhs=sb_xjb[:, (2 * p + 1) * 512 : (2 * p + 2) * 512],
                        start=True, stop=True,
                    )
                    h1 = h1pool.tile([128, 1024], DT.bfloat16,
                                     name=f"h1_{g}_{p}", tag="h1sb")
                    nc.scalar.activation(
                        out=h1, in_=h1ps, func=RELU, bias=sb_b1st, scale=1.0
                    )
                    h1tiles[p] = h1

                h2tiles = {}

                def make_h2(it):
                    h1 = h1tiles[it // 2][:, (it % 2) * 512 : (it % 2 + 1) * 512]
                    h2ps = ps_l2.tile([128, 2, 512], DT.float32, tag="l2",
                                      name=f"h2ps_{g}_{it}")
                    nc.tensor.matmul(
                        h2ps[:, 0, :], lhsT=sb_w2s[0:64, :], rhs=h1[0:64, :],
                        start=True, stop=True,
                    )
                    nc.tensor.matmul(
                        h2ps[:, 1, :], lhsT=sb_w2s[64:128, :], rhs=h1[64:128, :],
                        start=True, stop=True,
                    )
                    h2 = h2pool.tile([128, 1024], DT.bfloat16,
                                     name=f"h2_{g}_{it}", tag="h2sb")
                    nc.scalar.activation(
                        out=h2, in_=h2ps.rearrange("p a b -> p (a b)"),
                        func=RELU, bias=sb_b2c, scale=1.0,
                    )
                    h2tiles[it] = h2

                PP = 2
                for p in range(PP):
                    make_h1_pair(p)
                make_h2(0)

                # ---- main pipeline: per iteration, L2/relu for it+1 then
                # L3 + G-consumption for it (G streams via 2 psum banks) ----
                for it in range(NPAIR):
                    if it % 2 == 0 and it // 2 + PP < NPAIR // 2:
                        make_h1_pair(it // 2 + PP)
                    if it + 1 < NPAIR:
                        make_h2(it + 1)
                    h2 = h2tiles.pop(it)
                    for half, w3h in ((0, sb_w3a), (1, sb_w3b)):
                        gt = ps_g.tile([128, 1024], DT.float32, tag="g",
                                       name=f"gt_{g}_{it}_{half}")
                        nc.tensor.matmul(
                            gt[:, 0:512], lhsT=w3h, rhs=h2[:, 0:512],
                            start=True, stop=True,
                        )
                        nc.tensor.matmul(
                            gt[:, 512:1024], lhsT=w3h, rhs=h2[:, 512:1024],
                            start=True, stop=True,
                        )
                        t = it * 2 + half
                        if _copy_path(t):
                            if rbs[half] is None:
                                rb = raccpool.tile(
                                    [128, 1024], DT.float16, tag=f"rb{half}"
                                )
                                rbs[half] = rb
                                nc.scalar.copy(out=rb, in_=gt)
                            else:
                                gc = gcpool.tile([128, 1024], DT.float16)
                                nc.scalar.copy(out=gc, in_=gt)
                                nc.vector.tensor_tensor(
                                    out=rbs[half], in0=gc, in1=rbs[half],
                                    op=ALU.max,
                                )
                        else:
                            k = ridx[half]
                            ridx[half] += 1
                            nc.vector.reduce_max(
                                out=racc[:, half, k : k + 1], in_=gt, axis=AX.X
                            )

                # ---- P = max over accumulators, + b3; F MLP (N=1) ----
                pm1 = fmlp.tile([128, 2], DT.float32, tag="pm1")
                nc.vector.reduce_max(out=pm1, in_=racc, axis=AX.X)
                pm2 = fmlp.tile([128, 2], DT.float32, tag="pm2")
                for half in range(2):
                    nc.vector.reduce_max(
                        out=pm2[:, half : half + 1],
                        in_=rbs[half].rearrange("p (a b) -> p a b", a=1),
                        axis=AX.X,
                    )
                pmx = fmlp.tile([128, 2], DT.float32, tag="pmx")
                nc.vector.tensor_tensor(out=pmx, in0=pm1, in1=pm2, op=ALU.max)
                pb = fmlp.tile([128, 2], DT.float16, tag="pb")
                nc.vector.tensor_tensor(out=pb, in0=pmx, in1=sb_b3_2, op=ALU.add)

                y1ps = ps_l2.tile([128, 4], DT.float32, tag="l2",
                                  name=f"y1ps_{g}")
                for mm in range(4):
                    for kk in range(2):
                        nc.tensor.matmul(
                            y1ps[:, mm : mm + 1],
                            lhsT=v1t(kk)[:, mm * 128 : (mm + 1) * 128],
                            rhs=pb[:, kk : kk + 1],
                            start=(kk == 0),
                            stop=(kk == 1),
                        )
                y1pre = fmlp.tile([128, 4], DT.float32, tag="y1pre")
                nc.vector.tensor_tensor(out=y1pre, in0=y1ps, in1=sb_c1_4, op=ALU.add)
                y1 = fmlp.tile([128, 4], DT.float16, tag="y1")
                nc.vector.tensor_scalar_max(out=y1, in0=y1pre, scalar1=0.0)

                y2ps = ps_h1.tile([128, 2], DT.float32, tag="h1",
                                  name=f"y2ps_{g}")
                for mm in range(2):
                    for kk in range(4):
                        nc.tensor.matmul(
                            y2ps[:, mm : mm + 1],
                            lhsT=v2t(kk)[:, mm * 128 : (mm + 1) * 128],
                            rhs=y1[:, kk : kk + 1],
                            start=(kk == 0),
                            stop=(kk == 3),
                        )
                y2pre = fmlp.tile([128, 2], DT.float32, tag="y2pre")
                nc.vector.tensor_tensor(out=y2pre, in0=y2ps, in1=sb_c2_2, op=ALU.add)
                y2 = fmlp.tile([128, 2], DT.float16, tag="y2")
                nc.vector.tensor_scalar_max(out=y2, in0=y2pre, scalar1=0.0)

                y3ps = ps_g.tile([40, 1], DT.float32, tag="g",
                                 name=f"y3ps_{g}")
                for kk in range(2):
                    nc.tensor.matmul(
                        y3ps,
                        lhsT=v3t(kk)[:, 0:40],
                        rhs=y2[:, kk : kk + 1],
                        start=(kk == 0),
                        stop=(kk == 1),
                    )
                y3 = fmlp.tile([40, 1], DT.float32, tag="y3")
                nc.vector.tensor_scalar_add(out=y3, in0=y3ps, scalar1=sb_c3c)
                nc.sync.dma_start(out=y_out[g, :], in_=y3)

    _split_multi_waits(nc)
    return nc


# ---------------------------------------------------------------------------
# Host side
# ---------------------------------------------------------------------------
_NC_CACHE = None


def _get_program():
    global _NC_CACHE
    if _NC_CACHE is None:
        _NC_CACHE = _build_program()
    return _NC_CACHE


def _make_in_maps(inputs):
    X = np.asarray(inputs["X"], F32)
    W1 = np.asarray(inputs["W1"], F32)
    b1 = np.asarray(inputs["b1"], F32)
    W2 = np.asarray(inputs["W2"], F32)
    b2 = np.asarray(inputs["b2"], F32)
    W3 = np.asarray(inputs["W3"], F32)
    b3 = np.asarray(inputs["b3"], F32)
    V1 = np.asarray(inputs["V1"], F32)
    c1 = np.asarray(inputs["c1"], F32)
    V2 = np.asarray(inputs["V2"], F32)
    c2 = np.asarray(inputs["c2"], F32)
    V3 = np.asarray(inputs["V3"], F32)
    c3 = np.asarray(inputs["c3"], F32)

    W1A, W1B = W1[:, :DIN], W1[:, DIN:]
    z = np.zeros((DIN, 64), F32)
    w1blob = np.concatenate(
        [
            np.concatenate([W1A.T, z], axis=1),
            np.concatenate([z, W1A.T], axis=1),
            np.concatenate([W1B.T, W1B.T], axis=1),
        ],
        axis=0,
    ).astype(BF16)
    wblob = np.concatenate(
        [
            np.concatenate([W2.T, W2.T], axis=0),
            W3.T[:, 0:128],
            W3.T[:, 128:256],
        ],
        axis=1,
    ).astype(BF16)
    # v1t: V1.T is [256, 512] -> k-tiles stacked on cols [128, 2, 512]
    v1t_cols = V1.T.reshape(2, 128, 512).transpose(1, 0, 2).reshape(128, 1024)
    vblob = np.concatenate(
        [v1t_cols,
         V2.T.reshape(4, 128, 256).transpose(1, 0, 2).reshape(128, 1024),
         V3.T.reshape(2, 128, 40).transpose(1, 0, 2).reshape(128, 80)],
        axis=1,
    ).astype(np.float16)
    cblob = np.zeros((128, 11), F32)
    cblob[:, 0] = np.concatenate([b1, b1])
    cblob[:, 1] = b2
    cblob[:, 2:4] = b3.reshape(2, 128).T
    cblob[:, 4:8] = c1.reshape(4, 128).T
    cblob[:, 8:10] = c2.reshape(2, 128).T
    cblob[0:40, 10] = c3

    shared = dict(wblob=wblob, w1blob=w1blob, vblob=vblob, cblob=cblob)

    Xv = X.reshape(B, D, M, DIN)                    # (b, d, i, k)
    in_maps = []
    for c in range(N_CORES):
        xjbs = np.empty((GROUPS_PER_CORE, K9, NPAIR * 512), F32)
        for gi in range(GROUPS_PER_CORE):
            g = 2 * c + gi
            bb, dd = g // D, g % D
            xg = Xv[bb, dd]                          # (M, 3)
            # per jp block of 128 cols: rows 0:3 = x_{2jp}, 3:6 = x_{2jp+1}
            blk = xjbs[gi].reshape(K9, JP, M)
            blk[0:3] = xg[0::2].T[:, :, None]        # (3, JP, 1) -> bcast
            blk[3:6] = xg[1::2].T[:, :, None]
            blk[6:9] = xg.T[:, None, :]              # (3, M) tiled over jp
        in_maps.append(dict(shared, xjb=xjbs.astype(BF16)))
    return in_maps


def _run(inputs, trace=False):
    nc = _get_program()
    in_maps = _make_in_maps(inputs)
    res = run_bass_kernel_spmd(nc, in_maps, list(range(N_CORES)), trace=trace)
    ys = np.stack([res.results[c]["y"] for c in range(N_CORES)])  # [8, 2, 40]
    y16 = ys.reshape(B, D, 40)
    out = y16.max(axis=1).astype(F32)
    return out, res


def kernel(**inputs):
    out, _ = _run(inputs, trace=False)
    return out


# revision 19
# speedup vs baseline: 1.4996x; 1.0577x over previous
"""Trainium2 Bass kernel for BoostedPointPairNet2.

Model (per (b, d) group, m = 128 points, din = 3):
  H1(i,j) = relu(W1A @ x_j + W1B @ x_i + b1)          (64)
  H2(i,j) = relu(W2 @ H1 + b2)                        (128)
  G(i,j)  = W3 @ H2                                    (256, b3 deferred)
  P       = max_{i,j} G + b3                           (256)
  Y       = V3 @ relu(V2 @ relu(V1 @ P + c1) + c2) + c3  (40)
  out[b]  = max_d Y[b, d]

Sharding: 16 (b, d) groups over 8 cores, 2 groups per core. Weights
replicated. Each core returns its two groups' Y rows; the host does the
final max over d (the trivial "all-gather" of a (b, 40) output).

Per-core dataflow ("stacked pairs" layout): channels of two j-values are
stacked on the 128 SBUF partitions (j even -> 0-63, j odd -> 64-127).
Engine assignment per iteration (4 j-pairs = 1024 pairs):
  PE  : H1pre as ONE K=9 matmul [w19 = [W1A_e;W1A_o;W1B_2]] against a
        host-prebuilt 9-row rhs (x_j broadcasts + tiled x_i); L2 as a
        K=64 row-split pair (tile_position (0,0)/(64,0) run concurrently
        in the array: 2x512 cols in ~216ns); L3 into two [128,1024]
        G psum tiles (ch 0:128 / 128:256).
  ACT : relu-h1 [128,512], relu-h2 [128,1024], and a share of G copies
        to fp16 SBUF. ACTIVATE is strict 1x: (FD+352)/1.2 ns.
  DVE : direct reduce_max of G tiles from PSUM (1x) + running TT-max
        (2x fp16) of the ACT-copied tiles. Carries no H1 work at all.
The G split (ACT copy+TT vs DVE direct) is tuned so ACT and DVE finish
together; PE has slack. PSUM: one shared 4-slot ring of 2-bank tiles
(8 banks); each iteration allocates exactly 4 tiles (h1ps, h2ps, g0,
g1) so every kind gets 1-iteration double buffering.
"""

import numpy as np
import ml_dtypes

import bass_rust
import concourse.bass as bass
import concourse.mybir as mybir
from concourse.tile import TileContext
from concourse.bass_utils import run_bass_kernel_spmd

BF16 = ml_dtypes.bfloat16
F32 = np.float32
DT = mybir.dt
ALU = mybir.AluOpType
AX = mybir.AxisListType
RELU = mybir.ActivationFunctionType.Relu

N_CORES = 8
B, N, DIN = 4, 512, 3
D = 4                    # boost factor
M = N // D               # 128 points per group
GROUPS_PER_CORE = 2
JP = M // 2              # 64 stacked j-pairs per group
NPAIR = JP // 4          # 16 pipeline iterations per group (4 jp each)
K9 = 3 * DIN             # contraction dim of the fused L1 matmul

# Per-group G-tile path: t = it*2 + half in 0..31 (half: 0 = channels
# 0:128 via w3a, 1 = 128:256 via w3b; each G tile is [128, 1024] fp32).
# True -> ACT copy + DVE running TT-max; False -> DVE direct reduce_max.
# Balance: ACT = relu_h2 + relu_h1/2 + a*copy vs DVE = (64-a)*reduce +
# a*TT (per core)  ->  a ~= 16/64 -> 8/32 per group.
def _copy_path(t):
    return (t % 8) in (1, 4)


# ---------------------------------------------------------------------------
# Workaround: this walrus build accepts at most ONE sync wait per instruction
# ("Too many sync wait commands"), while Tile emits up to 3. Hoist extra
# waits onto same-engine nop instructions inserted just before the offender
# (engines execute their queue in order, so the AND-wait semantics hold).
# ---------------------------------------------------------------------------
def _split_multi_waits(nc):
    seq = 0
    for fn in nc.m.functions:
        for bb in fn.blocks:
            new = []
            changed = False
            for ins in bb.instructions:
                si = ins.sync_info
                waits = list(si.on_wait) if si is not None and si.on_wait else []
                if len(waits) > 1:
                    changed = True
                    for w in waits[:-1]:
                        seq += 1
                        new.append(
                            mybir.InstNoOp(
                                name=f"I-wsplit-{seq}",
                                engine=ins.engine,
                                sync_info=bass_rust.SyncInfo(
                                    on_wait=[w], on_update=[]
                                ),
                            )
                        )
                    ins.sync_info = bass_rust.SyncInfo(
                        on_wait=[waits[-1]], on_update=list(si.on_update or [])
                    )
                new.append(ins)
            if changed:
                bb.instructions = new


# ---------------------------------------------------------------------------
# Device program
# ---------------------------------------------------------------------------
def _build_program():
    nc = bass.Bass(
        "TRN2", target_bir_lowering=False, debug=False, num_devices=N_CORES
    )

    # 9-row rhs for the fused L1 matmul, one [9, 8192] block per group:
    # col block (it, jj) rows 0:3 = x_{2jp} (bcast), 3:6 = x_{2jp+1} (bcast),
    # 6:9 = X^T tiled (i varies along the 128 cols); jp = 4*it + jj.
    xjb = nc.declare_dram_parameter(
        "xjb", [GROUPS_PER_CORE, K9, NPAIR * 512], DT.bfloat16, isOutput=False
    )
    # col 0:128 w2stack=[W2T;W2T], 128:256 w3a=W3T[:,0:128], 256:384 w3b
    wblob = nc.declare_dram_parameter("wblob", [128, 384], DT.bfloat16, isOutput=False)
    # w19 = [[W1AT,0],[0,W1AT],[W1BT,W1BT]]  (9 x 128)
    w1blob = nc.declare_dram_parameter("w1blob", [K9, 128], DT.bfloat16, isOutput=False)
    # v1t (2x512) | v2t (4x256) | v3t (2x40)
    vblob = nc.declare_dram_parameter("vblob", [128, 2128], DT.float16, isOutput=False)
    # col 0 b1st, 1 b2c, 2:4 b3_2, 4:8 c1_4, 8:10 c2_2, 10 c3 (rows 0:40)
    cblob = nc.declare_dram_parameter("cblob", [128, 11], DT.float32, isOutput=False)
    y_out = nc.declare_dram_parameter(
        "y", [GROUPS_PER_CORE, 40], DT.float32, isOutput=True
    )

    # direct-reduce column count per half per group (for racc allocation);
    # racc is memset to -1e30 so unequal halves are safe.
    ndir = [0, 0]
    for it in range(NPAIR):
        for half in range(2):
            if not _copy_path(it * 2 + half):
                ndir[half] += 1
    ND = max(ndir)

    with TileContext(nc) as tc:
        with (
            tc.tile_pool(name="singles", bufs=1) as singles,
            tc.tile_pool(name="xjbp", bufs=1) as xjbp,
            tc.tile_pool(name="h1p", bufs=6) as h1pool,
            tc.tile_pool(name="h2p", bufs=4) as h2pool,
            tc.tile_pool(name="gcp", bufs=6) as gcpool,
            tc.tile_pool(name="raccp", bufs=2) as raccpool,
            tc.tile_pool(name="fmlp", bufs=8) as fmlp,
            tc.tile_pool(name="ps_h1", bufs=1, space="PSUM") as ps_h1,
            tc.tile_pool(name="ps_l2", bufs=1, space="PSUM") as ps_l2,
            tc.tile_pool(name="ps_g", bufs=2, space="PSUM") as ps_g,
        ):
            # ---- load inputs; issue order = need order (2 HWDGE queues) ----
            sb_xjbs = []
            for g in range(GROUPS_PER_CORE):
                sb_xjbs.append(
                    xjbp.tile([K9, NPAIR * 512], DT.bfloat16, name=f"sb_xjb{g}")
                )
            sb_w1 = singles.tile([K9, 128], DT.bfloat16, tag="w1blob")
            nc.sync.dma_start(out=sb_w1, in_=w1blob[:, :])
            sb_c = singles.tile([128, 11], DT.float32, tag="cblob")
            nc.gpsimd.dma_start(out=sb_c, in_=cblob[:, :])
            nc.sync.dma_start(out=sb_xjbs[0][:, 0:2048], in_=xjb[0][:, 0:2048])
            sb_w = singles.tile([128, 384], DT.bfloat16, tag="wblob")
            nc.gpsimd.dma_start(out=sb_w, in_=wblob[:, :])
            nc.sync.dma_start(out=sb_xjbs[0][:, 2048:], in_=xjb[0][:, 2048:])
            nc.gpsimd.dma_start(out=sb_xjbs[1], in_=xjb[1])
            sb_v = singles.tile([128, 2128], DT.float16, tag="vblob")
            nc.sync.dma_start(out=sb_v, in_=vblob[:, :])

            # dummy relu to hoist ACT_TABLE_LOAD (~1.3us) into the init shadow
            warm = singles.tile([1, 1], DT.float32, tag="warm")
            nc.vector.memset(warm, 0.0)
            nc.scalar.activation(out=warm, in_=warm, func=RELU)

            sb_w2s = sb_w[:, 0:128]
            sb_w3a, sb_w3b = sb_w[:, 128:256], sb_w[:, 256:384]
            sb_b1st = sb_c[:, 0:1]
            sb_b2c = sb_c[:, 1:2]
            sb_b3_2 = sb_c[:, 2:4]
            sb_c1_4 = sb_c[:, 4:8]
            sb_c2_2 = sb_c[:, 8:10]
            sb_c3c = sb_c[0:40, 10:11]

            def v1t(k):  # [128, 512] fp16, k in 0..1
                return sb_v[:, 512 * k : 512 * (k + 1)]

            def v2t(k):  # [128, 256] fp16, k in 0..3
                return sb_v[:, 1024 + 256 * k : 1024 + 256 * (k + 1)]

            def v3t(k):  # [128, 40] fp16, k in 0..1
                return sb_v[:, 2048 + 40 * k : 2048 + 40 * (k + 1)]

            # ---- PE warm-up burst: ~12 back-to-back MMs during the DMA
            # phase so HAM unthrottles (1.2 -> 2.4 GHz) before the pipeline.
            scratch = singles.tile([128, 512], DT.bfloat16, tag="scratch")
            nc.vector.memset(scratch, 0.0)
            wps = ps_l2.tile([128, 2, 512], DT.float32, tag="l2", name="wps")
            for rep in range(12):
                nc.tensor.matmul(
                    wps[:, rep % 2, :], lhsT=scratch[:, 0:128], rhs=scratch,
                    start=True, stop=True,
                )

            gstate = {}

            def pipeline(g):
                sb_xjb = sb_xjbs[g]
                racc = raccpool.tile([128, 2, ND], DT.float32,
                                     name=f"racc_{g}")
                nc.vector.memset(racc, -1e30)
                rbs = [None, None]           # running-max tiles (ch-lo, ch-hi)
                ridx = [0, 0]

                # h1 in 2-iteration pairs, prefetched PP pairs ahead; L2+relu
                # run one iteration ahead of L3/consume (software pipeline).
                h1tiles = {}

                def make_h1_pair(p):
                    h1ps = ps_h1.tile([128, 1024], DT.float32, tag="h1",
                                      name=f"h1ps_{g}_{p}")
                    nc.tensor.matmul(
                        h1ps[:, 0:512], lhsT=sb_w1,
                        rhs=sb_xjb[:, (2 * p) * 512 : (2 * p + 1) * 512],
                        start=True, stop=True,
                    )
                    nc.tensor.matmul(
                        h1ps[:, 512:1024], lhsT=sb_w1,
                        rhs=sb_xjb[:, (2 * p + 1) * 512 : (2 * p + 2) * 512],
                        start=True, stop=True,
                    )
                    h1 = h1pool.tile([128, 1024], DT.bfloat16,
                                     name=f"h1_{g}_{p}", tag="h1sb")
                    nc.scalar.activation(
                        out=h1, in_=h1ps, func=RELU, bias=sb_b1st, scale=1.0
                    )
                    h1tiles[p] = h1

                h2tiles = {}

                def make_h2(it):
                    h1 = h1tiles[it // 2][:, (it % 2) * 512 : (it % 2 + 1) * 512]
                    h2ps = ps_l2.tile([128, 2, 512], DT.float32, tag="l2",
                                      name=f"h2ps_{g}_{it}")
                    nc.tensor.matmul(
                        h2ps[:, 0, :], lhsT=sb_w2s[0:64, :], rhs=h1[0:64, :],
                        start=True, stop=True,
                    )
                    nc.tensor.matmul(
                        h2ps[:, 1, :], lhsT=sb_w2s[64:128, :], rhs=h1[64:128, :],
                        start=True, stop=True,
                    )
                    h2 = h2pool.tile([128, 1024], DT.bfloat16,
                                     name=f"h2_{g}_{it}", tag="h2sb")
                    nc.scalar.activation(
                        out=h2, in_=h2ps.rearrange("p a b -> p (a b)"),
                        func=RELU, bias=sb_b2c, scale=1.0,
                    )
                    h2tiles[it] = h2

                PP = 2
                for p in range(PP):
                    make_h1_pair(p)
                make_h2(0)

                # ---- main pipeline: per iteration, L2/relu for it+1 then
                # L3 + G-consumption for it (G streams via 2 psum banks) ----
                for it in range(NPAIR):
                    if it % 2 == 0 and it // 2 + PP < NPAIR // 2:
                        make_h1_pair(it // 2 + PP)
                    if it + 1 < NPAIR:
                        make_h2(it + 1)
                    h2 = h2tiles.pop(it)
                    for half, w3h in ((0, sb_w3a), (1, sb_w3b)):
                        gt = ps_g.tile([128, 1024], DT.float32, tag="g",
                                       name=f"gt_{g}_{it}_{half}")
                        nc.tensor.matmul(
                            gt[:, 0:512], lhsT=w3h, rhs=h2[:, 0:512],
                            start=True, stop=True,
                        )
                        nc.tensor.matmul(
                            gt[:, 512:1024], lhsT=w3h, rhs=h2[:, 512:1024],
                            start=True, stop=True,
                        )
                        t = it * 2 + half
                        if _copy_path(t):
                            if rbs[half] is None:
                                rb = raccpool.tile(
                                    [128, 1024], DT.float16, tag=f"rb{half}"
                                )
                                rbs[half] = rb
                                nc.scalar.copy(out=rb, in_=gt)
                            else:
                                gc = gcpool.tile([128, 1024], DT.float16)
                                nc.scalar.copy(out=gc, in_=gt)
                                nc.vector.tensor_tensor(
                                    out=rbs[half], in0=gc, in1=rbs[half],
                                    op=ALU.max,
                                )
                        else:
                            k = ridx[half]
                            ridx[half] += 1
                            nc.vector.reduce_max(
                                out=racc[:, half, k : k + 1], in_=gt, axis=AX.X
                            )

                gstate[g] = (racc, rbs)

            def fmlp_group(g):
                racc, rbs = gstate.pop(g)
                # ---- P = max over accumulators, + b3; F MLP (N=1) ----
                pm1 = fmlp.tile([128, 2], DT.float32, tag="pm1")
                nc.vector.reduce_max(out=pm1, in_=racc, axis=AX.X)
                pm2 = fmlp.tile([128, 2], DT.float32, tag="pm2")
                nc.vector.memset(pm2, -1e30)
                for half in range(2):
                    if rbs[half] is not None:
                        nc.vector.reduce_max(
                            out=pm2[:, half : half + 1],
                            in_=rbs[half].rearrange("p (a b) -> p a b", a=1),
                            axis=AX.X,
                        )
                pmx = fmlp.tile([128, 2], DT.float32, tag="pmx")
                nc.vector.tensor_tensor(out=pmx, in0=pm1, in1=pm2, op=ALU.max)
                pb = fmlp.tile([128, 2], DT.float16, tag="pb")
                nc.vector.tensor_tensor(out=pb, in0=pmx, in1=sb_b3_2, op=ALU.add)

                y1ps = ps_l2.tile([128, 4], DT.float32, tag="l2",
                                  name=f"y1ps_{g}")
                for mm in range(4):
                    for kk in range(2):
                        nc.tensor.matmul(
                            y1ps[:, mm : mm + 1],
                            lhsT=v1t(kk)[:, mm * 128 : (mm + 1) * 128],
                            rhs=pb[:, kk : kk + 1],
                            start=(kk == 0),
                            stop=(kk == 1),
                        )
                y1pre = fmlp.tile([128, 4], DT.float32, tag="y1pre")
                nc.vector.tensor_tensor(out=y1pre, in0=y1ps, in1=sb_c1_4, op=ALU.add)
                y1 = fmlp.tile([128, 4], DT.float16, tag="y1")
                nc.vector.tensor_scalar_max(out=y1, in0=y1pre, scalar1=0.0)

                y2ps = ps_h1.tile([128, 2], DT.float32, tag="h1",
                                  name=f"y2ps_{g}")
                for mm in range(2):
                    for kk in range(4):
                        nc.tensor.matmul(
                            y2ps[:, mm : mm + 1],
                            lhsT=v2t(kk)[:, mm * 128 : (mm + 1) * 128],
                            rhs=y1[:, kk : kk + 1],
                            start=(kk == 0),
                            stop=(kk == 3),
                        )
                y2pre = fmlp.tile([128, 2], DT.float32, tag="y2pre")
                nc.vector.tensor_tensor(out=y2pre, in0=y2ps, in1=sb_c2_2, op=ALU.add)
                y2 = fmlp.tile([128, 2], DT.float16, tag="y2")
                nc.vector.tensor_scalar_max(out=y2, in0=y2pre, scalar1=0.0)

                y3ps = ps_g.tile([40, 1], DT.float32, tag="g",
                                 name=f"y3ps_{g}")
                for kk in range(2):
                    nc.tensor.matmul(
                        y3ps,
                        lhsT=v3t(kk)[:, 0:40],
                        rhs=y2[:, kk : kk + 1],
                        start=(kk == 0),
                        stop=(kk == 1),
                    )
                y3 = fmlp.tile([40, 1], DT.float32, tag="y3")
                nc.vector.tensor_scalar_add(out=y3, in0=y3ps, scalar1=sb_c3c)
                nc.sync.dma_start(out=y_out[g, :], in_=y3)

            # group 1's pipeline is emitted before group 0's F-MLP so the
            # inter-group boundary keeps all engines (and HAM) busy.
            pipeline(0)
            pipeline(1)
            fmlp_group(0)
            fmlp_group(1)

    _split_multi_waits(nc)
    return nc


# ---------------------------------------------------------------------------
# Host side
# ---------------------------------------------------------------------------
_NC_CACHE = None


def _get_program():
    global _NC_CACHE
    if _NC_CACHE is None:
        _NC_CACHE = _build_program()
    return _NC_CACHE


def _make_in_maps(inputs):
    X = np.asarray(inputs["X"], F32)
    W1 = np.asarray(inputs["W1"], F32)
    b1 = np.asarray(inputs["b1"], F32)
    W2 = np.asarray(inputs["W2"], F32)
    b2 = np.asarray(inputs["b2"], F32)
    W3 = np.asarray(inputs["W3"], F32)
    b3 = np.asarray(inputs["b3"], F32)
    V1 = np.asarray(inputs["V1"], F32)
    c1 = np.asarray(inputs["c1"], F32)
    V2 = np.asarray(inputs["V2"], F32)
    c2 = np.asarray(inputs["c2"], F32)
    V3 = np.asarray(inputs["V3"], F32)
    c3 = np.asarray(inputs["c3"], F32)

    W1A, W1B = W1[:, :DIN], W1[:, DIN:]
    z = np.zeros((DIN, 64), F32)
    w1blob = np.concatenate(
        [
            np.concatenate([W1A.T, z], axis=1),
            np.concatenate([z, W1A.T], axis=1),
            np.concatenate([W1B.T, W1B.T], axis=1),
        ],
        axis=0,
    ).astype(BF16)
    wblob = np.concatenate(
        [
            np.concatenate([W2.T, W2.T], axis=0),
            W3.T[:, 0:128],
            W3.T[:, 128:256],
        ],
        axis=1,
    ).astype(BF16)
    # v1t: V1.T is [256, 512] -> k-tiles stacked on cols [128, 2, 512]
    v1t_cols = V1.T.reshape(2, 128, 512).transpose(1, 0, 2).reshape(128, 1024)
    vblob = np.concatenate(
        [v1t_cols,
         V2.T.reshape(4, 128, 256).transpose(1, 0, 2).reshape(128, 1024),
         V3.T.reshape(2, 128, 40).transpose(1, 0, 2).reshape(128, 80)],
        axis=1,
    ).astype(np.float16)
    cblob = np.zeros((128, 11), F32)
    cblob[:, 0] = np.concatenate([b1, b1])
    cblob[:, 1] = b2
    cblob[:, 2:4] = b3.reshape(2, 128).T
    cblob[:, 4:8] = c1.reshape(4, 128).T
    cblob[:, 8:10] = c2.reshape(2, 128).T
    cblob[0:40, 10] = c3

    shared = dict(wblob=wblob, w1blob=w1blob, vblob=vblob, cblob=cblob)

    Xv = X.reshape(B, D, M, DIN)                    # (b, d, i, k)
    in_maps = []
    for c in range(N_CORES):
        xjbs = np.empty((GROUPS_PER_CORE, K9, NPAIR * 512), F32)
        for gi in range(GROUPS_PER_CORE):
            g = 2 * c + gi
            bb, dd = g // D, g % D
            xg = Xv[bb, dd]                          # (M, 3)
            # per jp block of 128 cols: rows 0:3 = x_{2jp}, 3:6 = x_{2jp+1}
            blk = xjbs[gi].reshape(K9, JP, M)
            blk[0:3] = xg[0::2].T[:, :, None]        # (3, JP, 1) -> bcast
            blk[3:6] = xg[1::2].T[:, :, None]
            blk[6:9] = xg.T[:, None, :]              # (3, M) tiled over jp
        in_maps.append(dict(shared, xjb=xjbs.astype(BF16)))
    return in_maps


def _run(inputs, trace=False):
    nc = _get_program()
    in_maps = _make_in_maps(inputs)
    res = run_bass_kernel_spmd(nc, in_maps, list(range(N_CORES)), trace=trace)
    ys = np.stack([res.results[c]["y"] for c in range(N_CORES)])  # [8, 2, 40]
    y16 = ys.reshape(B, D, 40)
    out = y16.max(axis=1).astype(F32)
    return out, res


def kernel(**inputs):
    out, _ = _run(inputs, trace=False)
    return out
